# revision 4
# baseline (speedup 1.0000x reference)
import sys

sys.path.insert(0, "/opt/trn_rl_repo")
import numpy as np
import concourse.bass as bass
import concourse.mybir as mybir
from concourse.bass_utils import run_bass_kernel_spmd

NNODE = 500000
NELEM = 500000
NDOF = 2 * NNODE                 # 1000000
NPAD = 1000064                   # 128 * 7813
COLS = 7813
NCORES = 8
EPC = NELEM // NCORES            # 62500 elements per core
W = 128                          # windows per core (= partitions)
CAP = 512                        # element slots per window
KCOLS = CAP * 64                 # 32768 f32 per window row


def build_nc():
    f32 = mybir.dt.float32
    i32 = mybir.dt.int32
    nc = bass.Bass(target_bir_lowering=False)
    u_in = nc.dram_tensor("u_in", [128, COLS], f32, kind="ExternalInput")
    w_in = nc.dram_tensor("w_in", [128, COLS], f32, kind="ExternalInput")
    gidx = nc.dram_tensor("gidx", [128, W * 32], i32, kind="ExternalInput")
    sidx = nc.dram_tensor("sidx", [128, W * 32], i32, kind="ExternalInput")
    K_in = nc.dram_tensor("K_in", [128, KCOLS], f32, kind="ExternalInput")
    Fo = nc.dram_tensor("F_out", [NPAD, 1], f32, kind="ExternalOutput")
    Fo2 = nc.dram_tensor("F_out2", [NPAD, 1], f32, kind="ExternalOutput")
    u1d = nc.dram_tensor("u1d", [NPAD, 1], f32)  # Internal

    with (
        nc.Block() as block,
        nc.semaphore("uw_sem") as uw_sem,
        nc.semaphore("idx_sem") as idx_sem,
        nc.semaphore("u1_sem") as u1_sem,
        nc.semaphore("zf_sem") as zf_sem,
        nc.semaphore("gat_sem") as gat_sem,
        nc.semaphore("kb0_sem") as kb0_sem,
        nc.semaphore("kb1_sem") as kb1_sem,
        nc.semaphore("c_sem") as c_sem,
        nc.semaphore("sc0_sem") as sc0_sem,
        nc.semaphore("sc1_sem") as sc1_sem,
        nc.sbuf_tensor("u_t", [128, COLS], f32) as u_t,
        nc.sbuf_tensor("w_t", [128, COLS], f32) as w_t,
        nc.sbuf_tensor("gidx_t", [128, W * 32], i32) as gidx_t,
        nc.sbuf_tensor("sidx_t", [128, W * 32], i32) as sidx_t,
        nc.sbuf_tensor("ue_t", [128, 4096], f32) as ue_t,
        nc.sbuf_tensor("fe_t", [128, 4096], f32) as fe_t,
        nc.sbuf_tensor("tmp_t", [128, 4096], f32) as tmp_t,
        nc.sbuf_tensor("kb0", [128, 4096], f32) as kb0,
        nc.sbuf_tensor("kb1", [128, 4096], f32) as kb1,
    ):
        kbufs = [kb0, kb1]
        ksems = [kb0_sem, kb1_sem]

        @block.gpsimd
        def _(g):
            g.dma_start(out=u_t[:, :], in_=u_in[:, :]).then_inc(uw_sem, 16)
            g.dma_start(out=w_t[:, :], in_=w_in[:, :]).then_inc(uw_sem, 16)
            g.dma_start(out=gidx_t[:, :], in_=gidx[:, :]).then_inc(idx_sem, 16)
            g.dma_start(out=sidx_t[:, :], in_=sidx[:, :]).then_inc(idx_sem, 16)
            g.dma_start(out=kb0[:, :], in_=K_in[:, 0:4096]).then_inc(kb0_sem, 16)
            g.dma_start(out=kb1[:, :], in_=K_in[:, 4096:8192]).then_inc(kb1_sem, 16)

            # wait for DVE to finish u1 = u*w in-place in u_t
            g.wait_ge(c_sem, 1)
            g.dma_start(
                out=bass.AP(u1d, 0, [[COLS, 128], [1, COLS]]),
                in_=u_t[:, :],
            ).then_inc(u1_sem, 16)
            # w_t now free: reuse as zero tile to clear both accumulators
            g.memset(w_t[:, :], 0.0)
            g.dma_start(
                out=bass.AP(Fo, 0, [[COLS, 128], [1, COLS]]),
                in_=w_t[:, :],
            ).then_inc(zf_sem, 16)
            g.dma_start(
                out=bass.AP(Fo2, 0, [[COLS, 128], [1, COLS]]),
                in_=w_t[:, :],
            ).then_inc(zf_sem, 16)

            g.wait_ge(u1_sem, 16)
            g.wait_ge(idx_sem, 32)
            # 512 gather windows of 1024 descriptors: window (jslot, p)
            # fills ue_t[p, 1024*jslot : 1024*(jslot+1)]. Small windows
            # run ~3x faster per descriptor than 4096-desc ones.
            for jslot in range(4):
                for p in range(W):
                    w = jslot * W + p
                    g.indirect_dma_start(
                        out=bass.AP(ue_t, p * 4096 + 1024 * jslot,
                                    [[4096, 1], [1, 1024], [1, 1]]),
                        out_offset=None,
                        in_=u1d[:, :],
                        in_offset=bass.IndirectOffsetOnAxis(
                            ap=gidx_t[:, 8 * w:8 * w + 8], axis=0),
                    ).then_inc(gat_sem, 16)

            for c in range(2, 8):
                g.wait_ge(c_sem, c)  # DVE done with chunk c-2 -> buffer free
                g.dma_start(
                    out=kbufs[c % 2][:, :],
                    in_=K_in[:, 4096 * c:4096 * (c + 1)],
                ).then_inc(ksems[c % 2], 16)

            g.wait_ge(c_sem, 9)   # all fe chunks computed
            g.wait_ge(zf_sem, 32)
            # even windows -> Fo, odd -> Fo2: same-buffer scatters serialize
            # (cross-instruction RMW on same dof) but the two chains overlap
            for w in range(W):
                tgt = Fo if w % 2 == 0 else Fo2
                ssem = sc0_sem if w % 2 == 0 else sc1_sem
                if w >= 2:
                    g.wait_ge(ssem, 16 * (w // 2))
                g.indirect_dma_start(
                    out=tgt[:, :],
                    out_offset=bass.IndirectOffsetOnAxis(
                        ap=sidx_t[:, 32 * w:32 * w + 32], axis=0),
                    in_=bass.AP(fe_t, w * 4096, [[4096, 1], [1, 4096], [1, 1]]),
                    in_offset=None,
                    compute_op=mybir.AluOpType.add,
                ).then_inc(ssem, 16)
            g.wait_ge(sc0_sem, 16 * (W // 2))
            g.wait_ge(sc1_sem, 16 * (W // 2))

        @block.vector
        def _(v):
            v.wait_ge(uw_sem, 32)
            v.tensor_mul(u_t[:, :], u_t[:, :], w_t[:, :]).then_inc(c_sem, 1)
            for c in range(8):
                v.wait_ge(ksems[c % 2], 16 * (c // 2 + 1))
                if c == 0:
                    v.wait_ge(gat_sem, 16 * 4 * W)
                buf = kbufs[c % 2]
                for i in range(8):
                    v.tensor_mul(
                        bass.AP(tmp_t, 8 * i, [[4096, 128], [64, 64], [1, 8]]),
                        bass.AP(buf, 8 * i, [[4096, 128], [64, 64], [1, 8]]),
                        bass.AP(ue_t, 512 * c, [[4096, 128], [8, 64], [1, 8]]),
                    )
                v.tensor_reduce(
                    out=bass.AP(fe_t, 512 * c, [[4096, 128], [1, 512]]),
                    in_=bass.AP(tmp_t, 0, [[4096, 128], [8, 512], [1, 8]]),
                    axis=mybir.AxisListType.X,
                    op=mybir.AluOpType.add,
                ).then_inc(c_sem, 1)

    return nc


def _make_copies(ed):
    """Split elements with internally-duplicated dofs into copies with
    disjoint active-slot masks so every active dof in a copy is unique."""
    E = ed.shape[0]
    srt = np.sort(ed, axis=1)
    hasdup = (srt[:, 1:] == srt[:, :-1]).any(axis=1)
    simple = np.nonzero(~hasdup)[0]
    celem = [simple]
    cmask = [np.ones((simple.size, 8), dtype=bool)]
    for e in np.nonzero(hasdup)[0]:
        row = ed[e]
        groups = {}
        for s in range(8):
            groups.setdefault(int(row[s]), []).append(s)
        m = max(len(v) for v in groups.values())
        masks = np.zeros((m, 8), dtype=bool)
        for slots in groups.values():
            for r, s in enumerate(slots):
                masks[r, s] = True
        celem.append(np.full(m, e, dtype=np.int64))
        cmask.append(masks)
    return np.concatenate(celem), np.concatenate(cmask, axis=0)


def _color(cdof, cmask):
    """Assign each copy a (window, slot) so no window contains two active
    descriptors targeting the same dof. Vectorized greedy rounds."""
    n = cdof.shape[0]
    assert n <= W * CAP
    occupied = np.zeros(W * NPAD, dtype=bool)
    wcount = np.zeros(W, dtype=np.int64)
    w = np.arange(n, dtype=np.int64) % W
    win_out = np.empty(n, dtype=np.int64)
    slot_out = np.empty(n, dtype=np.int64)
    rem = np.arange(n)
    rounds = 0
    while rem.size:
        rounds += 1
        assert rounds < 1000, "coloring failed to converge"
        ww = w[rem]
        kk = ww[:, None] * NPAD + cdof[rem]
        mk = cmask[rem]
        occ = np.zeros(kk.shape, dtype=bool)
        occ[mk] = occupied[kk[mk]]
        ok_occ = ~occ.any(axis=1)
        kflat = np.where(
            mk, kk, -1 - np.arange(kk.size, dtype=np.int64).reshape(kk.shape))
        _, fi = np.unique(kflat.ravel(), return_index=True)
        isf = np.zeros(kk.size, dtype=bool)
        isf[fi] = True
        ok = ok_occ & isf.reshape(kk.shape).all(axis=1)
        cand = np.nonzero(ok)[0]
        acc_local = np.zeros(rem.size, dtype=bool)
        if cand.size:
            cw = ww[cand]
            order = np.argsort(cw, kind="stable")
            cs = cw[order]
            start = np.searchsorted(cs, np.arange(W))
            rank = np.arange(cs.size) - start[cs]
            cap_ok = rank < (CAP - wcount)[cs]
            acc_sorted = cand[order][cap_ok]
            acc_w = cs[cap_ok]
            acc_slot = (wcount[cs] + rank)[cap_ok]
            gids = rem[acc_sorted]
            win_out[gids] = acc_w
            slot_out[gids] = acc_slot
            akk = acc_w[:, None] * NPAD + cdof[gids]
            am = cmask[gids]
            occupied[akk[am]] = True
            wcount += np.bincount(acc_w, minlength=W)
            acc_local[acc_sorted] = True
        new_rem = rem[~acc_local]
        w[new_rem] = (w[new_rem] + 1) % W
        rem = new_rem
    return win_out, slot_out


def preprocess_core(ed, stiff):
    celem, cmask = _make_copies(ed)
    cdof = ed[celem]                       # (n, 8) int64
    win, slot = _color(cdof, cmask)

    garr = np.zeros((W, CAP, 8), dtype=np.int32)
    sarr = np.full((W, CAP, 8), NDOF, dtype=np.int32)   # pad target
    Karr = np.zeros((W, CAP, 8, 8), dtype=np.float32)
    garr[win, slot] = cdof.astype(np.int32)
    sarr[win, slot] = np.where(cmask, cdof, NDOF).astype(np.int32)
    Karr[win, slot] = stiff[celem]

    def pack(a):
        # instr w consumes desc k <- tile[k % 128, 32*w + k // 128]
        return np.ascontiguousarray(
            a.reshape(W, 32, 128).transpose(2, 0, 1).reshape(128, W * 32))

    def pack_g(a):
        # gather window w=(jslot,p) covers ue_t[p, 1024*jslot + k]; its
        # DMA consumes desc k <- tile[k % 128, 8*w + k // 128]
        dev = np.empty((128, W * 32), np.int32)
        for jslot in range(4):
            for p in range(W):
                w = jslot * W + p
                blk = np.ascontiguousarray(
                    a[p, 1024 * jslot:1024 * (jslot + 1)])
                dev[:, 8 * w:8 * (w + 1)] = blk.reshape(8, 128).T
        return dev

    gidx_dev = pack_g(garr.reshape(W, CAP * 8))
    sidx_dev = pack(sarr.reshape(W, CAP * 8))
    Kdev = np.ascontiguousarray(Karr.reshape(W, KCOLS))
    return gidx_dev, sidx_dev, Kdev


def make_in_maps(u, weight1, edof, stiffness):
    upad = np.zeros(NPAD, dtype=np.float32)
    upad[:NDOF] = np.asarray(u, dtype=np.float32)
    wpad = np.zeros(NPAD, dtype=np.float32)
    wpad[:NDOF] = np.asarray(weight1, dtype=np.float32)
    u2d = upad.reshape(128, COLS)
    w2d = wpad.reshape(128, COLS)
    edof = np.asarray(edof, dtype=np.int64)
    stiffness = np.asarray(stiffness, dtype=np.float32)
    in_maps = []
    for k in range(NCORES):
        ed = edof[EPC * k:EPC * (k + 1)]
        st = stiffness[EPC * k:EPC * (k + 1)]
        gdev, sdev, Kdev = preprocess_core(ed, st)
        in_maps.append({"u_in": u2d, "w_in": w2d, "gidx": gdev,
                        "sidx": sdev, "K_in": Kdev})
    return in_maps


def kernel(u, weight1, bc_idx, edof, stiffness):
    # bc_idx is arange(NDOF) (all dofs free) -> u1 = weight1 * u elementwise
    in_maps = make_in_maps(u, weight1, edof, stiffness)
    nc = build_nc()
    res = run_bass_kernel_spmd(nc, in_maps, list(range(NCORES)))
    F = np.zeros(NPAD, dtype=np.float32)
    for r in res.results:
        F += r["F_out"].reshape(-1)
        F += r["F_out2"].reshape(-1)
    return F[:NDOF].astype(np.float32)



# revision 5
# speedup vs baseline: 1.0057x; 1.0057x over previous
import sys

sys.path.insert(0, "/opt/trn_rl_repo")
import numpy as np
import concourse.bass as bass
import concourse.mybir as mybir
from concourse.bass_utils import run_bass_kernel_spmd

NNODE = 500000
NELEM = 500000
NDOF = 2 * NNODE                 # 1000000
NPAD = 1000064                   # 128 * 7813
COLS = 7813
NCORES = 8
EPC = NELEM // NCORES            # 62500 elements per core
W = 128                          # windows per core (= partitions)
CAP = 512                        # element slots per window
KCOLS = CAP * 64                 # 32768 f32 per window row


def build_nc():
    f32 = mybir.dt.float32
    i32 = mybir.dt.int32
    nc = bass.Bass(target_bir_lowering=False)
    u_in = nc.dram_tensor("u_in", [128, COLS], f32, kind="ExternalInput")
    w_in = nc.dram_tensor("w_in", [128, COLS], f32, kind="ExternalInput")
    gidx = nc.dram_tensor("gidx", [128, W * 32], i32, kind="ExternalInput")
    sidx = nc.dram_tensor("sidx", [128, W * 32], i32, kind="ExternalInput")
    K_in = nc.dram_tensor("K_in", [128, KCOLS], f32, kind="ExternalInput")
    Fo = nc.dram_tensor("F_out", [NPAD, 1], f32, kind="ExternalOutput")
    Fo2 = nc.dram_tensor("F_out2", [NPAD, 1], f32, kind="ExternalOutput")
    u1d = nc.dram_tensor("u1d", [NPAD, 1], f32)  # Internal

    with (
        nc.Block() as block,
        nc.semaphore("uw_sem") as uw_sem,
        nc.semaphore("idx_sem") as idx_sem,
        nc.semaphore("u1_sem") as u1_sem,
        nc.semaphore("zf_sem") as zf_sem,
        nc.semaphore("gat_sem") as gat_sem,
        nc.semaphore("kb0_sem") as kb0_sem,
        nc.semaphore("kb1_sem") as kb1_sem,
        nc.semaphore("c_sem") as c_sem,
        nc.semaphore("sc0_sem") as sc0_sem,
        nc.semaphore("sc1_sem") as sc1_sem,
        nc.sbuf_tensor("u_t", [128, COLS], f32) as u_t,
        nc.sbuf_tensor("w_t", [128, COLS], f32) as w_t,
        nc.sbuf_tensor("gidx_t", [128, W * 32], i32) as gidx_t,
        nc.sbuf_tensor("sidx_t", [128, W * 32], i32) as sidx_t,
        nc.sbuf_tensor("ue_t", [128, 4096], f32) as ue_t,
        nc.sbuf_tensor("fe_t", [128, 4096], f32) as fe_t,
        nc.sbuf_tensor("tmp_t", [128, 4096], f32) as tmp_t,
        nc.sbuf_tensor("kb0", [128, 4096], f32) as kb0,
        nc.sbuf_tensor("kb1", [128, 4096], f32) as kb1,
    ):
        kbufs = [kb0, kb1]
        ksems = [kb0_sem, kb1_sem]

        @block.gpsimd
        def _(g):
            g.dma_start(out=u_t[:, :], in_=u_in[:, :]).then_inc(uw_sem, 16)
            g.dma_start(out=w_t[:, :], in_=w_in[:, :]).then_inc(uw_sem, 16)
            g.dma_start(out=gidx_t[:, :], in_=gidx[:, :]).then_inc(idx_sem, 16)
            g.dma_start(out=sidx_t[:, :], in_=sidx[:, :]).then_inc(idx_sem, 16)
            g.dma_start(out=kb0[:, :], in_=K_in[:, 0:4096]).then_inc(kb0_sem, 16)
            g.dma_start(out=kb1[:, :], in_=K_in[:, 4096:8192]).then_inc(kb1_sem, 16)

            # wait for DVE to finish u1 = u*w in-place in u_t
            g.wait_ge(c_sem, 1)
            g.dma_start(
                out=bass.AP(u1d, 0, [[COLS, 128], [1, COLS]]),
                in_=u_t[:, :],
            ).then_inc(u1_sem, 16)
            # w_t now free: reuse as zero tile to clear both accumulators
            g.memset(w_t[:, :], 0.0)
            g.dma_start(
                out=bass.AP(Fo, 0, [[COLS, 128], [1, COLS]]),
                in_=w_t[:, :],
            ).then_inc(zf_sem, 16)
            g.dma_start(
                out=bass.AP(Fo2, 0, [[COLS, 128], [1, COLS]]),
                in_=w_t[:, :],
            ).then_inc(zf_sem, 16)

            g.wait_ge(u1_sem, 16)
            g.wait_ge(idx_sem, 32)
            for w in range(W):
                g.indirect_dma_start(
                    out=bass.AP(ue_t, w * 4096, [[4096, 1], [1, 4096], [1, 1]]),
                    out_offset=None,
                    in_=u1d[:, :],
                    in_offset=bass.IndirectOffsetOnAxis(
                        ap=gidx_t[:, 32 * w:32 * w + 32], axis=0),
                ).then_inc(gat_sem, 16)

            for c in range(2, 8):
                g.wait_ge(c_sem, c)  # DVE done with chunk c-2 -> buffer free
                g.dma_start(
                    out=kbufs[c % 2][:, :],
                    in_=K_in[:, 4096 * c:4096 * (c + 1)],
                ).then_inc(ksems[c % 2], 16)

            g.wait_ge(c_sem, 9)   # all fe chunks computed
            g.wait_ge(zf_sem, 32)
            # even windows -> Fo, odd -> Fo2: same-buffer scatters serialize
            # (cross-instruction RMW on same dof) but the two chains overlap
            for w in range(W):
                tgt = Fo if w % 2 == 0 else Fo2
                ssem = sc0_sem if w % 2 == 0 else sc1_sem
                if w >= 2:
                    g.wait_ge(ssem, 16 * (w // 2))
                g.indirect_dma_start(
                    out=tgt[:, :],
                    out_offset=bass.IndirectOffsetOnAxis(
                        ap=sidx_t[:, 32 * w:32 * w + 32], axis=0),
                    in_=bass.AP(fe_t, w * 4096, [[4096, 1], [1, 4096], [1, 1]]),
                    in_offset=None,
                    compute_op=mybir.AluOpType.add,
                ).then_inc(ssem, 16)
            g.wait_ge(sc0_sem, 16 * (W // 2))
            g.wait_ge(sc1_sem, 16 * (W // 2))

        @block.vector
        def _(v):
            v.wait_ge(uw_sem, 32)
            v.tensor_mul(u_t[:, :], u_t[:, :], w_t[:, :]).then_inc(c_sem, 1)
            for c in range(8):
                v.wait_ge(ksems[c % 2], 16 * (c // 2 + 1))
                if c == 0:
                    v.wait_ge(gat_sem, 16 * W)
                buf = kbufs[c % 2]
                for i in range(8):
                    v.tensor_mul(
                        bass.AP(tmp_t, 8 * i, [[4096, 128], [64, 64], [1, 8]]),
                        bass.AP(buf, 8 * i, [[4096, 128], [64, 64], [1, 8]]),
                        bass.AP(ue_t, 512 * c, [[4096, 128], [8, 64], [1, 8]]),
                    )
                v.tensor_reduce(
                    out=bass.AP(fe_t, 512 * c, [[4096, 128], [1, 512]]),
                    in_=bass.AP(tmp_t, 0, [[4096, 128], [8, 512], [1, 8]]),
                    axis=mybir.AxisListType.X,
                    op=mybir.AluOpType.add,
                ).then_inc(c_sem, 1)

    return nc


def _make_copies(ed):
    """Split elements with internally-duplicated dofs into copies with
    disjoint active-slot masks so every active dof in a copy is unique."""
    E = ed.shape[0]
    srt = np.sort(ed, axis=1)
    hasdup = (srt[:, 1:] == srt[:, :-1]).any(axis=1)
    simple = np.nonzero(~hasdup)[0]
    celem = [simple]
    cmask = [np.ones((simple.size, 8), dtype=bool)]
    for e in np.nonzero(hasdup)[0]:
        row = ed[e]
        groups = {}
        for s in range(8):
            groups.setdefault(int(row[s]), []).append(s)
        m = max(len(v) for v in groups.values())
        masks = np.zeros((m, 8), dtype=bool)
        for slots in groups.values():
            for r, s in enumerate(slots):
                masks[r, s] = True
        celem.append(np.full(m, e, dtype=np.int64))
        cmask.append(masks)
    return np.concatenate(celem), np.concatenate(cmask, axis=0)


def _color(cdof, cmask):
    """Assign each copy a (window, slot) so no window contains two active
    descriptors targeting the same dof. Vectorized greedy rounds."""
    n = cdof.shape[0]
    assert n <= W * CAP
    occupied = np.zeros(W * NPAD, dtype=bool)
    wcount = np.zeros(W, dtype=np.int64)
    w = np.arange(n, dtype=np.int64) % W
    win_out = np.empty(n, dtype=np.int64)
    slot_out = np.empty(n, dtype=np.int64)
    rem = np.arange(n)
    rounds = 0
    while rem.size:
        rounds += 1
        assert rounds < 1000, "coloring failed to converge"
        ww = w[rem]
        kk = ww[:, None] * NPAD + cdof[rem]
        mk = cmask[rem]
        occ = np.zeros(kk.shape, dtype=bool)
        occ[mk] = occupied[kk[mk]]
        ok_occ = ~occ.any(axis=1)
        kflat = np.where(
            mk, kk, -1 - np.arange(kk.size, dtype=np.int64).reshape(kk.shape))
        _, fi = np.unique(kflat.ravel(), return_index=True)
        isf = np.zeros(kk.size, dtype=bool)
        isf[fi] = True
        ok = ok_occ & isf.reshape(kk.shape).all(axis=1)
        cand = np.nonzero(ok)[0]
        acc_local = np.zeros(rem.size, dtype=bool)
        if cand.size:
            cw = ww[cand]
            order = np.argsort(cw, kind="stable")
            cs = cw[order]
            start = np.searchsorted(cs, np.arange(W))
            rank = np.arange(cs.size) - start[cs]
            cap_ok = rank < (CAP - wcount)[cs]
            acc_sorted = cand[order][cap_ok]
            acc_w = cs[cap_ok]
            acc_slot = (wcount[cs] + rank)[cap_ok]
            gids = rem[acc_sorted]
            win_out[gids] = acc_w
            slot_out[gids] = acc_slot
            akk = acc_w[:, None] * NPAD + cdof[gids]
            am = cmask[gids]
            occupied[akk[am]] = True
            wcount += np.bincount(acc_w, minlength=W)
            acc_local[acc_sorted] = True
        new_rem = rem[~acc_local]
        w[new_rem] = (w[new_rem] + 1) % W
        rem = new_rem
    return win_out, slot_out


def preprocess_core(ed, stiff):
    celem, cmask = _make_copies(ed)
    cdof = ed[celem]                       # (n, 8) int64
    win, slot = _color(cdof, cmask)

    garr = np.zeros((W, CAP, 8), dtype=np.int32)
    sarr = np.full((W, CAP, 8), NDOF, dtype=np.int32)   # pad target
    Karr = np.zeros((W, CAP, 8, 8), dtype=np.float32)
    garr[win, slot] = cdof.astype(np.int32)
    sarr[win, slot] = np.where(cmask, cdof, NDOF).astype(np.int32)
    Karr[win, slot] = stiff[celem]

    def pack(a):
        # instr w consumes desc k <- tile[k % 128, 32*w + k // 128]
        return np.ascontiguousarray(
            a.reshape(W, 32, 128).transpose(2, 0, 1).reshape(128, W * 32))

    gidx_dev = pack(garr.reshape(W, CAP * 8))
    sidx_dev = pack(sarr.reshape(W, CAP * 8))
    Kdev = np.ascontiguousarray(Karr.reshape(W, KCOLS))
    return gidx_dev, sidx_dev, Kdev


def make_in_maps(u, weight1, edof, stiffness):
    upad = np.zeros(NPAD, dtype=np.float32)
    upad[:NDOF] = np.asarray(u, dtype=np.float32)
    wpad = np.zeros(NPAD, dtype=np.float32)
    wpad[:NDOF] = np.asarray(weight1, dtype=np.float32)
    u2d = upad.reshape(128, COLS)
    w2d = wpad.reshape(128, COLS)
    edof = np.asarray(edof, dtype=np.int64)
    stiffness = np.asarray(stiffness, dtype=np.float32)
    in_maps = []
    for k in range(NCORES):
        ed = edof[EPC * k:EPC * (k + 1)]
        st = stiffness[EPC * k:EPC * (k + 1)]
        gdev, sdev, Kdev = preprocess_core(ed, st)
        in_maps.append({"u_in": u2d, "w_in": w2d, "gidx": gdev,
                        "sidx": sdev, "K_in": Kdev})
    return in_maps


def kernel(u, weight1, bc_idx, edof, stiffness):
    # bc_idx is arange(NDOF) (all dofs free) -> u1 = weight1 * u elementwise
    in_maps = make_in_maps(u, weight1, edof, stiffness)
    nc = build_nc()
    res = run_bass_kernel_spmd(nc, in_maps, list(range(NCORES)))
    F = np.zeros(NPAD, dtype=np.float32)
    for r in res.results:
        F += r["F_out"].reshape(-1)
        F += r["F_out2"].reshape(-1)
    return F[:NDOF].astype(np.float32)

